# revision 32
# baseline (speedup 1.0000x reference)
"""Single-head causal self-attention on 8 Trainium2 NeuronCores.

Problem: x[8, 4096, 1024], Wq/Wk/Wv[1024, 128] ->
  out[b] = softmax(causal((x[b] @ Wq) @ (x[b] @ Wk)^T / sqrt(128))) @ (x[b] @ Wv)

Sharding: data-parallel over batch -- each of the 8 cores handles one batch
element (xT = x[b].T fed per-core so the contraction dim C is on partitions).

Per-core kernel (T=4096, C=1024, HS=128), fp16 operands everywhere (more
mantissa than bf16 and unlocks DVE 2x mode for the fp16 accumulator adds):

  Phase 1 (QKV): x fully SBUF-resident (64KB/partition), DMA'd in 8 t-chunks
    so the first matmul starts ~2.5us. qT,kT [d,T] via W-stationary matmuls;
    v PE-transposed into natural [t,d] blocks.
  Phase 2 (attention), scores TRANSPOSED [kv, q], q-groups of 1024:
    - PSUM: 3-slot score ring (6 banks) + o_ps (2 banks).
    - exp on ScalarE, one slot per instruction, trimmed to the causal range.
    - causal masking of the diagonal 128-block via a PE accumulate-matmul
      (-60000*I @ strict-upper mask) added into the score PSUM -- exp then
      yields exact zeros, no DVE masking.
    - denominator: fp16 running acc += pt on DVE at 2x mode; per-group
      ones-matmul reduction.
    - epilogue entirely inside o_ps's own banks: ones-matmul denominator ->
      f32 reciprocal (DVE) -> fp16 cast -> PE broadcast matmul (ones x recip)
      -> one TT multiply. No DRAM round-trip.
  Scalar activation table preloaded with a dummy exp during the initial DMAs.
"""

import numpy as np

import concourse.bass as bass
import concourse.tile as tile
from concourse import bacc, mybir
from concourse.bass_utils import run_bass_kernel_spmd

B, T, C, HS = 8, 4096, 1024, 128
P = 128
NCORES = 8
CCH = C // P            # 8 c-chunks
NT = T // P             # 32 kv blocks of 128
TG = T // 512           # 8 t-groups of 512 (phase 1)
QG = T // 1024          # 4 q-groups of 1024 (phase 2)
SCALE = float(HS) ** -0.5
NEG = -60000.0          # large negative representable in fp16

f32 = mybir.dt.float32
f16 = mybir.dt.float16
EXP = mybir.ActivationFunctionType.Exp

_NC = None

# aux fp16 layout: [ones(130) | ident(128) | triU(128) | combo(256)]
AUX_W = 130 + 128 + 128 + 256


def build_aux() -> np.ndarray:
    aux = np.zeros((P, AUX_W), dtype=np.float16)
    aux[:, 0:130] = 1.0
    aux[:, 130:258] = np.eye(P, dtype=np.float16)
    # triU[c, q] = 1 where c > q  (kv > q within the diagonal 128-block)
    tri = np.tril(np.ones((P, P), dtype=np.float16), -1)
    aux[:, 258:386] = tri
    # combo mask for the odd block of a diagonal pair: 128 all-invalid
    # prefix cols followed by its own 128-wide triangle
    aux[:, 386:514] = 1.0
    aux[:, 514:642] = tri
    return aux


def build_program():
    nc = bacc.Bacc()
    xT = nc.declare_dram_parameter("xT", [C, T], f16, isOutput=False)
    # weights pre-packed on host to [p, i, j, d] so one contiguous DMA loads
    # all three projections
    Wall = nc.declare_dram_parameter("Wall", [P, 3 * CCH * HS], f16,
                                     isOutput=False)
    aux = nc.declare_dram_parameter("aux", [P, AUX_W], f16, isOutput=False)
    outT = nc.declare_dram_parameter("outT", [HS, T], f32, isOutput=True)

    xT_r = xT[:].rearrange("(j p) t -> p j t", p=P)

    with tile.TileContext(nc) as tc:
        with (
            tc.tile_pool(name="consts", bufs=1) as consts,
            tc.tile_pool(name="big", bufs=1) as big,
        ):
            # DMA issue order is the startup critical path: weights first
            # (first matmul's stationary), then the first t-group of x, then
            # the constants, then the rest of x.
            wall_sb = consts.tile([P, 3, CCH, HS], f16, tag="w", name="w")
            wall_v = Wall[:].rearrange("p (i j d) -> p i j d", i=3, j=CCH)
            for i in range(3):
                nc.sync.dma_start(out=wall_sb[:, i, :, :],
                                  in_=wall_v[:, i, :, :])
            w_sb = [wall_sb[:, i, :, :] for i in range(3)]

            x_sb = big.tile([P, CCH, T], f16, tag="x")
            for j in range(CCH):
                nc.sync.dma_start(out=x_sb[:, j, 0:512],
                                  in_=xT_r[:, j, 0:512])

            aux_sb = consts.tile([P, AUX_W], f16)
            nc.sync.dma_start(out=aux_sb[:], in_=aux[:])
            ones_col = aux_sb[:, 0:1]          # [128,1] dr lhsT
            ones_row = aux_sb[0:1, 0:128]      # [1,128] bcast lhsT
            ident = aux_sb[:, 130:258]         # transpose identity
            triU = aux_sb[:, 258:386]          # strict upper (kv>q) mask
            combo = aux_sb[:, 386:642]         # prefix + triangle, 256 wide

            # -60000 * I for PE-side causal masking (scaled from ident)
            identN = consts.tile([P, P], f16, tag="identN", name="identN")
            nc.vector.tensor_scalar_mul(identN[:], ident, NEG)

            # preload exp table while DMAs run
            warm = consts.tile([1, 2], f16, tag="warm", name="warm")
            nc.scalar.activation(warm[:], aux_sb[0:1, 0:2], EXP)

            qT = big.tile([P, T], f16, tag="qT")       # [d, t]
            kT = big.tile([P, T], f16, tag="kT")       # [d, t]
            vS = big.tile([P, NT, HS], f16, tag="vS")  # [t-in-block, blk, d]

            # ---------------- Phase 1: QKV projections ----------------
            with (
                tc.tile_pool(name="vtp", bufs=2) as vtp,
                tc.tile_pool(name="ps_qkv", bufs=2, space="PSUM") as ps_qkv,
                tc.tile_pool(name="ps_tr", bufs=2, space="PSUM") as ps_tr,
            ):
                for tg in range(1, TG):
                    t0 = 512 * tg
                    nc.sync.dma_start(out=x_sb[:, :, t0:t0 + 512],
                                      in_=xT_r[:, :, t0:t0 + 512])
                for tg in range(TG):
                    t0 = 512 * tg
                    ps3 = [ps_qkv.tile([P, 512], f32, tag=f"ps{i}",
                                       name=f"ps{i}") for i in range(3)]
                    for j in range(CCH):
                        for i in range(3):
                            nc.tensor.matmul(
                                ps3[i][:], lhsT=w_sb[i][:, j, :],
                                rhs=x_sb[:, j, t0:t0 + 512],
                                start=(j == 0), stop=(j == CCH - 1),
                            )
                    # all copies on DVE: ScalarE must stay free for phase-2
                    # exps that overlap the phase-1 tail
                    nc.vector.tensor_copy(qT[:, t0:t0 + 512], ps3[0][:])
                    nc.vector.tensor_copy(kT[:, t0:t0 + 512], ps3[1][:])
                    vt = vtp.tile([P, 512], f16)
                    nc.vector.tensor_copy(vt[:], ps3[2][:])
                    for m in range(4):
                        tp = ps_tr.tile([P, P], f16)
                        nc.tensor.transpose(
                            tp[:], vt[:, 128 * m:128 * (m + 1)], ident)
                        nc.vector.tensor_copy(vS[:, 4 * tg + m, :], tp[:])

            # ---------------- Phase 2: causal attention ----------------
            with (
                tc.tile_pool(name="ptp", bufs=6) as ptp,
                tc.tile_pool(name="accp", bufs=2) as accp,
                tc.tile_pool(name="ocup", bufs=2) as ocup,
                tc.tile_pool(name="recipp", bufs=2) as recipp,
                tc.tile_pool(name="ocnp", bufs=2) as ocnp,
                tc.tile_pool(name="ring", bufs=3, space="PSUM") as ring,
                tc.tile_pool(name="ps_o", bufs=1, space="PSUM") as ps_o,
            ):
                def emit_score(g, k, dst, d0):
                    """Score matmuls for kv block k into dst cols
                    [d0, d0+1024); returns causal col start."""
                    q0 = 1024 * g
                    va = max(0, 128 * k - q0)
                    for c in range(2):
                        cq = 512 * c
                        lc = max(0, va - cq)
                        if lc >= 512:
                            continue
                        nc.tensor.matmul(
                            dst[:, d0 + cq + lc:d0 + cq + 512],
                            lhsT=kT[:, 128 * k:128 * (k + 1)],
                            rhs=qT[:, q0 + cq + lc:q0 + cq + 512],
                            start=True, stop=True,
                        )
                    return va

                def emit_mask(g, k, dst, d0, va):
                    if k < 8 * g:
                        return
                    nc.tensor.matmul(
                        dst[:, d0 + va:d0 + va + 128],
                        lhsT=identN[:], rhs=triU,
                        start=False, stop=True, skip_group_check=True,
                    )

                def emit_pv(g, k, pt, d0, va, o_ps):
                    for c in range(2):
                        cq = 512 * c
                        lc = max(0, va - cq)
                        if lc >= 512:
                            continue
                        nc.tensor.matmul(
                            o_ps[:, cq + lc:cq + 512],
                            lhsT=vS[:, k, :],
                            rhs=pt[:, d0 + cq + lc:d0 + cq + 512],
                            start=(k == 0), stop=(k == 8 * g + 4 * c + 3),
                        )

                def emit_add(pt, d0, va, acc, first):
                    if first:
                        nc.vector.tensor_copy(acc[:], pt[:, d0:d0 + 1024])
                    else:
                        nc.vector.tensor_add(
                            acc[:, va:1024], acc[:, va:1024],
                            pt[:, d0 + va:d0 + 1024])

                for g in range(QG):
                    q0 = 1024 * g
                    o_ps = ps_o.tile([P, 1024], f32)
                    acc = accp.tile([P, 1024], f16, tag="acc", name="acc")
                    nkv = 8 * (g + 1)
                    for k in range(nkv):
                        sT = ring.tile([P, 1024], f32, tag="s", name="s")
                        va = emit_score(g, k, sT, 0)
                        emit_mask(g, k, sT, 0, va)
                        pt = ptp.tile([P, 1024], f16, tag="pt", name="pt")
                        nc.scalar.activation(
                            pt[:, va:1024], sT[:, va:1024], EXP, scale=SCALE)
                        emit_add(pt, 0, va, acc, k == 0)
                        emit_pv(g, k, pt, 0, va, o_ps)

                    # ---- epilogue: two independent 512-col chains (one per
                    # PSUM bank) so the final group's tail pipelines; o_ps
                    # freed half by half, denominator chain in a ring slot
                    ocu = ocup.tile([P, 1024], f32, tag="ocu", name="ocu")
                    ep = ring.tile([P, 1024], f32, tag="s", name="s")
                    recipT = recipp.tile([1, 1024], f32, tag="recipT",
                                         name="recipT")
                    recipH = recipp.tile([1, 1024], f16, tag="recipH",
                                         name="recipH")
                    ocn = ocnp.tile([P, 1024], f32, tag="ocn", name="ocn")
                    for c in range(2):
                        cs = slice(512 * c, 512 * (c + 1))
                        nc.vector.tensor_copy(ocu[:, cs], o_ps[:, cs])
                        nc.tensor.matmul(
                            ep[0:1, cs], lhsT=ones_col, rhs=acc[:, cs],
                            start=True, stop=True,
                        )
                        nc.vector.reciprocal_approx_fast(
                            recipT[0:1, cs], ep[0:1, cs])
                        nc.vector.tensor_copy(recipH[0:1, cs],
                                              recipT[0:1, cs])
                        nc.tensor.matmul(
                            ep[:, cs], lhsT=ones_row, rhs=recipH[0:1, cs],
                            start=True, stop=True,
                        )
                        nc.vector.tensor_mul(ocn[:, cs], ocu[:, cs],
                                             ep[:, cs])
                        nc.sync.dma_start(out=outT[:, q0 + 512 * c:
                                                   q0 + 512 * (c + 1)],
                                          in_=ocn[:, cs])

    nc.finalize()
    return nc


def _get_nc():
    global _NC
    if _NC is None:
        _NC = build_program()
    return _NC


def make_in_maps(x, Wq, Wk, Wv):
    xh = np.asarray(x, dtype=np.float32).astype(np.float16)
    # pack [C, HS] x3 -> [p, i, j, d]: Wall[p, i, j, :] = W_i[j*128+p, :]
    ws = np.stack([np.asarray(w, dtype=np.float32).astype(np.float16)
                   for w in (Wq, Wk, Wv)])            # [3, C, HS]
    wall = np.ascontiguousarray(
        ws.reshape(3, CCH, P, HS).transpose(2, 0, 1, 3).reshape(P, -1))
    aux = build_aux()
    return [
        {
            "xT": np.ascontiguousarray(xh[b].T),
            "Wall": wall,
            "aux": aux,
        }
        for b in range(NCORES)
    ]


def kernel(x, Wq, Wk, Wv):
    assert x.shape == (B, T, C) and Wq.shape == (C, HS)
    nc = _get_nc()
    in_maps = make_in_maps(x, Wq, Wk, Wv)
    res = run_bass_kernel_spmd(nc, in_maps, list(range(NCORES)))
    return np.stack([np.ascontiguousarray(res.results[b]["outT"].T)
                     for b in range(NCORES)])


# revision 35
# speedup vs baseline: 1.0259x; 1.0259x over previous
"""Single-head causal self-attention on 8 Trainium2 NeuronCores.

Problem: x[8, 4096, 1024], Wq/Wk/Wv[1024, 128] ->
  out[b] = softmax(causal((x[b] @ Wq) @ (x[b] @ Wk)^T / sqrt(128))) @ (x[b] @ Wv)

Sharding: data-parallel over batch -- each of the 8 cores handles one batch
element (xT = x[b].T fed per-core so the contraction dim C is on partitions).

Per-core kernel (T=4096, C=1024, HS=128), fp16 operands everywhere (more
mantissa than bf16 and unlocks DVE 2x mode for the fp16 accumulator adds):

  Phase 1 (QKV): x fully SBUF-resident (64KB/partition), DMA'd in 8 t-chunks
    so the first matmul starts ~2.5us. qT,kT [d,T] via W-stationary matmuls;
    v PE-transposed into natural [t,d] blocks.
  Phase 2 (attention), scores TRANSPOSED [kv, q], q-groups of 1024:
    - PSUM: 3-slot score ring (6 banks) + o_ps (2 banks).
    - exp on ScalarE, one slot per instruction, trimmed to the causal range.
    - causal masking of the diagonal 128-block via a PE accumulate-matmul
      (-60000*I @ strict-upper mask) added into the score PSUM -- exp then
      yields exact zeros, no DVE masking.
    - denominator: fp16 running acc += pt on DVE at 2x mode; per-group
      ones-matmul reduction.
    - epilogue entirely inside o_ps's own banks: ones-matmul denominator ->
      f32 reciprocal (DVE) -> fp16 cast -> PE broadcast matmul (ones x recip)
      -> one TT multiply. No DRAM round-trip.
  Scalar activation table preloaded with a dummy exp during the initial DMAs.
"""

import numpy as np

import concourse.bass as bass
import concourse.tile as tile
from concourse import bacc, mybir
from concourse.bass_utils import run_bass_kernel_spmd

B, T, C, HS = 8, 4096, 1024, 128
P = 128
NCORES = 8
CCH = C // P            # 8 c-chunks
NT = T // P             # 32 kv blocks of 128
TG = T // 512           # 8 t-groups of 512 (phase 1)
QG = T // 1024          # 4 q-groups of 1024 (phase 2)
SCALE = float(HS) ** -0.5
NEG = -60000.0          # large negative representable in fp16

f32 = mybir.dt.float32
f16 = mybir.dt.float16
EXP = mybir.ActivationFunctionType.Exp

_NC = None

# aux fp16 layout: [ones(130) | ident(128) | triU(128) | combo(256)]
AUX_W = 130 + 128 + 128 + 256


def build_aux() -> np.ndarray:
    aux = np.zeros((P, AUX_W), dtype=np.float16)
    aux[:, 0:130] = 1.0
    aux[:, 130:258] = np.eye(P, dtype=np.float16)
    # triU[c, q] = 1 where c > q  (kv > q within the diagonal 128-block)
    tri = np.tril(np.ones((P, P), dtype=np.float16), -1)
    aux[:, 258:386] = tri
    # combo mask for the odd block of a diagonal pair: 128 all-invalid
    # prefix cols followed by its own 128-wide triangle
    aux[:, 386:514] = 1.0
    aux[:, 514:642] = tri
    return aux


def build_program():
    nc = bacc.Bacc()
    xT = nc.declare_dram_parameter("xT", [C, T], f16, isOutput=False)
    # weights pre-packed on host to [p, i, j, d] so one contiguous DMA loads
    # all three projections
    Wall = nc.declare_dram_parameter("Wall", [P, 3 * CCH * HS], f16,
                                     isOutput=False)
    aux = nc.declare_dram_parameter("aux", [P, AUX_W], f16, isOutput=False)
    outT = nc.declare_dram_parameter("outT", [HS, T], f32, isOutput=True)

    xT_r = xT[:].rearrange("(j p) t -> p j t", p=P)

    with tile.TileContext(nc) as tc:
        with (
            tc.tile_pool(name="consts", bufs=1) as consts,
            tc.tile_pool(name="big", bufs=1) as big,
        ):
            # DMA issue order is the startup critical path: weights first
            # (first matmul's stationary), then the first t-group of x, then
            # the constants, then the rest of x.
            wall_sb = consts.tile([P, 3, CCH, HS], f16, tag="w", name="w")
            wall_v = Wall[:].rearrange("p (i j d) -> p i j d", i=3, j=CCH)
            for i in range(3):
                nc.sync.dma_start(out=wall_sb[:, i, :, :],
                                  in_=wall_v[:, i, :, :])
            w_sb = [wall_sb[:, i, :, :] for i in range(3)]

            x_sb = big.tile([P, CCH, T], f16, tag="x")
            for j in range(CCH):
                nc.sync.dma_start(out=x_sb[:, j, 0:512],
                                  in_=xT_r[:, j, 0:512])

            aux_sb = consts.tile([P, AUX_W], f16)
            nc.sync.dma_start(out=aux_sb[:], in_=aux[:])
            ones_col = aux_sb[:, 0:1]          # [128,1] dr lhsT
            ones_row = aux_sb[0:1, 0:128]      # [1,128] bcast lhsT
            ident = aux_sb[:, 130:258]         # transpose identity
            triU = aux_sb[:, 258:386]          # strict upper (kv>q) mask
            combo = aux_sb[:, 386:642]         # prefix + triangle, 256 wide

            # -60000 * I for PE-side causal masking (scaled from ident)
            identN = consts.tile([P, P], f16, tag="identN", name="identN")
            nc.vector.tensor_scalar_mul(identN[:], ident, NEG)

            # preload exp table while DMAs run
            warm = consts.tile([1, 2], f16, tag="warm", name="warm")
            nc.scalar.activation(warm[:], aux_sb[0:1, 0:2], EXP)

            qT = big.tile([P, T], f16, tag="qT")       # [d, t]
            kT = big.tile([P, T], f16, tag="kT")       # [d, t]
            vS = big.tile([P, NT, HS], f16, tag="vS")  # [t-in-block, blk, d]

            # ---------------- Phase 1: QKV projections ----------------
            with (
                tc.tile_pool(name="vtp", bufs=2) as vtp,
                tc.tile_pool(name="ps_qkv", bufs=2, space="PSUM") as ps_qkv,
                tc.tile_pool(name="ps_tr", bufs=2, space="PSUM") as ps_tr,
            ):
                for tg in range(1, TG):
                    t0 = 512 * tg
                    nc.sync.dma_start(out=x_sb[:, :, t0:t0 + 512],
                                      in_=xT_r[:, :, t0:t0 + 512])
                for tg in range(TG):
                    t0 = 512 * tg
                    ps3 = [ps_qkv.tile([P, 512], f32, tag=f"ps{i}",
                                       name=f"ps{i}") for i in range(3)]
                    for j in range(CCH):
                        for i in range(3):
                            nc.tensor.matmul(
                                ps3[i][:], lhsT=w_sb[i][:, j, :],
                                rhs=x_sb[:, j, t0:t0 + 512],
                                start=(j == 0), stop=(j == CCH - 1),
                            )
                    # all copies on DVE: ScalarE must stay free for phase-2
                    # exps that overlap the phase-1 tail
                    nc.vector.tensor_copy(qT[:, t0:t0 + 512], ps3[0][:])
                    nc.vector.tensor_copy(kT[:, t0:t0 + 512], ps3[1][:])
                    vt = vtp.tile([P, 512], f16)
                    nc.vector.tensor_copy(vt[:], ps3[2][:])
                    for m in range(4):
                        tp = ps_tr.tile([P, P], f16)
                        nc.tensor.transpose(
                            tp[:], vt[:, 128 * m:128 * (m + 1)], ident)
                        nc.vector.tensor_copy(vS[:, 4 * tg + m, :], tp[:])

            # ---------------- Phase 2: causal attention ----------------
            with (
                tc.tile_pool(name="ptp", bufs=6) as ptp,
                tc.tile_pool(name="accp", bufs=2) as accp,
                tc.tile_pool(name="ocup", bufs=2) as ocup,
                tc.tile_pool(name="recipp", bufs=2) as recipp,
                tc.tile_pool(name="ocnp", bufs=2) as ocnp,
                tc.tile_pool(name="ring", bufs=3, space="PSUM") as ring,
                tc.tile_pool(name="ps_o", bufs=1, space="PSUM") as ps_o,
            ):
                def emit_score(g, k, dst, d0):
                    """Score matmuls for kv block k into dst cols
                    [d0, d0+1024); returns causal col start."""
                    q0 = 1024 * g
                    va = max(0, 128 * k - q0)
                    for c in range(2):
                        cq = 512 * c
                        lc = max(0, va - cq)
                        if lc >= 512:
                            continue
                        nc.tensor.matmul(
                            dst[:, d0 + cq + lc:d0 + cq + 512],
                            lhsT=kT[:, 128 * k:128 * (k + 1)],
                            rhs=qT[:, q0 + cq + lc:q0 + cq + 512],
                            start=True, stop=True,
                        )
                    return va

                def emit_mask(g, k, dst, d0, va):
                    if k < 8 * g:
                        return
                    nc.tensor.matmul(
                        dst[:, d0 + va:d0 + va + 128],
                        lhsT=identN[:], rhs=triU,
                        start=False, stop=True, skip_group_check=True,
                    )

                def emit_pv(g, k, pt, d0, va, o_ps):
                    for c in range(2):
                        cq = 512 * c
                        lc = max(0, va - cq)
                        if lc >= 512:
                            continue
                        nc.tensor.matmul(
                            o_ps[:, cq + lc:cq + 512],
                            lhsT=vS[:, k, :],
                            rhs=pt[:, d0 + cq + lc:d0 + cq + 512],
                            start=(k == 0), stop=(k == 8 * g + 4 * c + 3),
                        )

                def emit_add(pt, d0, va, acc, first):
                    if first:
                        nc.vector.tensor_copy(acc[:], pt[:, d0:d0 + 1024])
                    else:
                        nc.vector.tensor_add(
                            acc[:, va:1024], acc[:, va:1024],
                            pt[:, d0 + va:d0 + 1024])

                for g in range(QG):
                    q0 = 1024 * g
                    o_ps = ps_o.tile([P, 1024], f32)
                    acc = accp.tile([P, 1024], f16, tag="acc", name="acc")
                    nkv = 8 * (g + 1)
                    for k in range(nkv):
                        sT = ring.tile([P, 1024], f32, tag="s", name="s")
                        va = emit_score(g, k, sT, 0)
                        emit_mask(g, k, sT, 0, va)
                        pt = ptp.tile([P, 1024], f16, tag="pt", name="pt")
                        nc.scalar.activation(
                            pt[:, va:1024], sT[:, va:1024], EXP, scale=SCALE)
                        emit_add(pt, 0, va, acc, k == 0)
                        emit_pv(g, k, pt, 0, va, o_ps)

                    # ---- epilogue: free o_ps ASAP (copy out on DVE), then
                    # run the denominator chain in a ring slot off the
                    # critical path. The last group's chain IS the kernel
                    # tail, so only there it runs as two pipelined 512-col
                    # half-chains (one per PSUM bank); mid-group epilogues
                    # stay single-shot to avoid extra boundary overhead.
                    ocu = ocup.tile([P, 1024], f32, tag="ocu", name="ocu")
                    ep = ring.tile([P, 1024], f32, tag="s", name="s")
                    recipT = recipp.tile([1, 1024], f32, tag="recipT",
                                         name="recipT")
                    recipH = recipp.tile([1, 1024], f16, tag="recipH",
                                         name="recipH")
                    ocn = ocnp.tile([P, 1024], f32, tag="ocn", name="ocn")
                    halves = ((slice(0, 512), slice(512, 1024))
                              if g == QG - 1 else (slice(0, 1024),))
                    if g != QG - 1:
                        nc.vector.tensor_copy(ocu[:], o_ps[:])
                    for cs in halves:
                        if g == QG - 1:
                            nc.vector.tensor_copy(ocu[:, cs], o_ps[:, cs])
                        for c in range(cs.start // 512,
                                       (cs.stop + 511) // 512):
                            nc.tensor.matmul(
                                ep[0:1, 512 * c:512 * (c + 1)],
                                lhsT=ones_col,
                                rhs=acc[:, 512 * c:512 * (c + 1)],
                                start=True, stop=True,
                            )
                        nc.vector.reciprocal_approx_fast(
                            recipT[0:1, cs], ep[0:1, cs])
                        nc.vector.tensor_copy(recipH[0:1, cs],
                                              recipT[0:1, cs])
                        for c in range(cs.start // 512,
                                       (cs.stop + 511) // 512):
                            nc.tensor.matmul(
                                ep[:, 512 * c:512 * (c + 1)],
                                lhsT=ones_row,
                                rhs=recipH[0:1, 512 * c:512 * (c + 1)],
                                start=True, stop=True,
                            )
                        nc.vector.tensor_mul(ocn[:, cs], ocu[:, cs],
                                             ep[:, cs])
                        nc.sync.dma_start(
                            out=outT[:, q0 + cs.start:q0 + cs.stop],
                            in_=ocn[:, cs])

    nc.finalize()
    return nc


def _get_nc():
    global _NC
    if _NC is None:
        _NC = build_program()
    return _NC


def make_in_maps(x, Wq, Wk, Wv):
    xh = np.asarray(x, dtype=np.float32).astype(np.float16)
    # pack [C, HS] x3 -> [p, i, j, d]: Wall[p, i, j, :] = W_i[j*128+p, :]
    ws = np.stack([np.asarray(w, dtype=np.float32).astype(np.float16)
                   for w in (Wq, Wk, Wv)])            # [3, C, HS]
    wall = np.ascontiguousarray(
        ws.reshape(3, CCH, P, HS).transpose(2, 0, 1, 3).reshape(P, -1))
    aux = build_aux()
    return [
        {
            "xT": np.ascontiguousarray(xh[b].T),
            "Wall": wall,
            "aux": aux,
        }
        for b in range(NCORES)
    ]


def kernel(x, Wq, Wk, Wv):
    assert x.shape == (B, T, C) and Wq.shape == (C, HS)
    nc = _get_nc()
    in_maps = make_in_maps(x, Wq, Wk, Wv)
    res = run_bass_kernel_spmd(nc, in_maps, list(range(NCORES)))
    return np.stack([np.ascontiguousarray(res.results[b]["outT"].T)
                     for b in range(NCORES)])


# revision 36
# speedup vs baseline: 1.0359x; 1.0097x over previous
"""Single-head causal self-attention on 8 Trainium2 NeuronCores.

Problem: x[8, 4096, 1024], Wq/Wk/Wv[1024, 128] ->
  out[b] = softmax(causal((x[b] @ Wq) @ (x[b] @ Wk)^T / sqrt(128))) @ (x[b] @ Wv)

Sharding: data-parallel over batch -- each of the 8 cores handles one batch
element (xT = x[b].T fed per-core so the contraction dim C is on partitions).

Per-core kernel (T=4096, C=1024, HS=128), fp16 operands everywhere (more
mantissa than bf16 and unlocks DVE 2x mode for the fp16 accumulator adds):

  Phase 1 (QKV): x fully SBUF-resident (64KB/partition), DMA'd in 8 t-chunks
    so the first matmul starts ~2.5us. qT,kT [d,T] via W-stationary matmuls;
    v PE-transposed into natural [t,d] blocks.
  Phase 2 (attention), scores TRANSPOSED [kv, q], q-groups of 1024:
    - PSUM: 3-slot score ring (6 banks) + o_ps (2 banks).
    - exp on ScalarE, one slot per instruction, trimmed to the causal range.
    - causal masking of the diagonal 128-block via a PE accumulate-matmul
      (-60000*I @ strict-upper mask) added into the score PSUM -- exp then
      yields exact zeros, no DVE masking.
    - denominator: fp16 running acc += pt on DVE at 2x mode; per-group
      ones-matmul reduction.
    - epilogue entirely inside o_ps's own banks: ones-matmul denominator ->
      f32 reciprocal (DVE) -> fp16 cast -> PE broadcast matmul (ones x recip)
      -> one TT multiply. No DRAM round-trip.
  Scalar activation table preloaded with a dummy exp during the initial DMAs.
"""

import numpy as np

import concourse.bass as bass
import concourse.tile as tile
from concourse import bacc, mybir
from concourse.bass_utils import run_bass_kernel_spmd

B, T, C, HS = 8, 4096, 1024, 128
P = 128
NCORES = 8
CCH = C // P            # 8 c-chunks
NT = T // P             # 32 kv blocks of 128
TG = T // 512           # 8 t-groups of 512 (phase 1)
QG = T // 1024          # 4 q-groups of 1024 (phase 2)
SCALE = float(HS) ** -0.5
NEG = -60000.0          # large negative representable in fp16

f32 = mybir.dt.float32
f16 = mybir.dt.float16
EXP = mybir.ActivationFunctionType.Exp

_NC = None

# aux fp16 layout: [ones(130) | ident(128) | triU(128) | combo(256)]
AUX_W = 130 + 128 + 128 + 256


def build_aux() -> np.ndarray:
    aux = np.zeros((P, AUX_W), dtype=np.float16)
    aux[:, 0:130] = 1.0
    aux[:, 130:258] = np.eye(P, dtype=np.float16)
    # triU[c, q] = 1 where c > q  (kv > q within the diagonal 128-block)
    tri = np.tril(np.ones((P, P), dtype=np.float16), -1)
    aux[:, 258:386] = tri
    # combo mask for the odd block of a diagonal pair: 128 all-invalid
    # prefix cols followed by its own 128-wide triangle
    aux[:, 386:514] = 1.0
    aux[:, 514:642] = tri
    return aux


def build_program():
    nc = bacc.Bacc()
    xT = nc.declare_dram_parameter("xT", [C, T], f16, isOutput=False)
    # weights pre-packed on host to [p, i, j, d] so one contiguous DMA loads
    # all three projections
    Wall = nc.declare_dram_parameter("Wall", [P, 3 * CCH * HS], f16,
                                     isOutput=False)
    aux = nc.declare_dram_parameter("aux", [P, AUX_W], f16, isOutput=False)
    outT = nc.declare_dram_parameter("outT", [HS, T], f32, isOutput=True)

    xT_r = xT[:].rearrange("(j p) t -> p j t", p=P)

    with tile.TileContext(nc) as tc:
        with (
            tc.tile_pool(name="consts", bufs=1) as consts,
            tc.tile_pool(name="big", bufs=1) as big,
        ):
            # DMA issue order is the startup critical path: weights first
            # (first matmul's stationary), then the first t-group of x, then
            # the constants, then the rest of x.
            wall_sb = consts.tile([P, 3, CCH, HS], f16, tag="w", name="w")
            nc.sync.dma_start(
                out=wall_sb[:],
                in_=Wall[:].rearrange("p (i j d) -> p i j d", i=3, j=CCH))
            w_sb = [wall_sb[:, i, :, :] for i in range(3)]

            x_sb = big.tile([P, CCH, T], f16, tag="x")
            for j in range(CCH):
                nc.sync.dma_start(out=x_sb[:, j, 0:512],
                                  in_=xT_r[:, j, 0:512])

            aux_sb = consts.tile([P, AUX_W], f16)
            nc.sync.dma_start(out=aux_sb[:], in_=aux[:])
            ones_col = aux_sb[:, 0:1]          # [128,1] dr lhsT
            ones_row = aux_sb[0:1, 0:128]      # [1,128] bcast lhsT
            ident = aux_sb[:, 130:258]         # transpose identity
            triU = aux_sb[:, 258:386]          # strict upper (kv>q) mask
            combo = aux_sb[:, 386:642]         # prefix + triangle, 256 wide

            # -60000 * I for PE-side causal masking (scaled from ident)
            identN = consts.tile([P, P], f16, tag="identN", name="identN")
            nc.vector.tensor_scalar_mul(identN[:], ident, NEG)

            # preload exp table while DMAs run
            warm = consts.tile([1, 2], f16, tag="warm", name="warm")
            nc.scalar.activation(warm[:], aux_sb[0:1, 0:2], EXP)

            qT = big.tile([P, T], f16, tag="qT")       # [d, t]
            kT = big.tile([P, T], f16, tag="kT")       # [d, t]
            vS = big.tile([P, NT, HS], f16, tag="vS")  # [t-in-block, blk, d]

            # ---------------- Phase 1: QKV projections ----------------
            with (
                tc.tile_pool(name="vtp", bufs=2) as vtp,
                tc.tile_pool(name="ps_qkv", bufs=2, space="PSUM") as ps_qkv,
                tc.tile_pool(name="ps_tr", bufs=2, space="PSUM") as ps_tr,
            ):
                for tg in range(1, TG):
                    t0 = 512 * tg
                    nc.sync.dma_start(out=x_sb[:, :, t0:t0 + 512],
                                      in_=xT_r[:, :, t0:t0 + 512])
                for tg in range(TG):
                    t0 = 512 * tg
                    ps3 = [ps_qkv.tile([P, 512], f32, tag=f"ps{i}",
                                       name=f"ps{i}") for i in range(3)]
                    for j in range(CCH):
                        for i in range(3):
                            nc.tensor.matmul(
                                ps3[i][:], lhsT=w_sb[i][:, j, :],
                                rhs=x_sb[:, j, t0:t0 + 512],
                                start=(j == 0), stop=(j == CCH - 1),
                            )
                    # all copies on DVE: ScalarE must stay free for phase-2
                    # exps that overlap the phase-1 tail
                    nc.vector.tensor_copy(qT[:, t0:t0 + 512], ps3[0][:])
                    nc.vector.tensor_copy(kT[:, t0:t0 + 512], ps3[1][:])
                    vt = vtp.tile([P, 512], f16)
                    nc.vector.tensor_copy(vt[:], ps3[2][:])
                    for m in range(4):
                        tp = ps_tr.tile([P, P], f16)
                        nc.tensor.transpose(
                            tp[:], vt[:, 128 * m:128 * (m + 1)], ident)
                        nc.vector.tensor_copy(vS[:, 4 * tg + m, :], tp[:])

            # ---------------- Phase 2: causal attention ----------------
            with (
                tc.tile_pool(name="ptp", bufs=6) as ptp,
                tc.tile_pool(name="accp", bufs=2) as accp,
                tc.tile_pool(name="ocup", bufs=2) as ocup,
                tc.tile_pool(name="recipp", bufs=2) as recipp,
                tc.tile_pool(name="ocnp", bufs=2) as ocnp,
                tc.tile_pool(name="ring", bufs=3, space="PSUM") as ring,
                tc.tile_pool(name="ps_o", bufs=1, space="PSUM") as ps_o,
            ):
                def emit_score(g, k, dst, d0):
                    """Score matmuls for kv block k into dst cols
                    [d0, d0+1024); returns causal col start."""
                    q0 = 1024 * g
                    va = max(0, 128 * k - q0)
                    for c in range(2):
                        cq = 512 * c
                        lc = max(0, va - cq)
                        if lc >= 512:
                            continue
                        nc.tensor.matmul(
                            dst[:, d0 + cq + lc:d0 + cq + 512],
                            lhsT=kT[:, 128 * k:128 * (k + 1)],
                            rhs=qT[:, q0 + cq + lc:q0 + cq + 512],
                            start=True, stop=True,
                        )
                    return va

                def emit_mask(g, k, dst, d0, va):
                    if k < 8 * g:
                        return
                    nc.tensor.matmul(
                        dst[:, d0 + va:d0 + va + 128],
                        lhsT=identN[:], rhs=triU,
                        start=False, stop=True, skip_group_check=True,
                    )

                def emit_pv(g, k, pt, d0, va, o_ps):
                    for c in range(2):
                        cq = 512 * c
                        lc = max(0, va - cq)
                        if lc >= 512:
                            continue
                        nc.tensor.matmul(
                            o_ps[:, cq + lc:cq + 512],
                            lhsT=vS[:, k, :],
                            rhs=pt[:, d0 + cq + lc:d0 + cq + 512],
                            start=(k == 0), stop=(k == 8 * g + 4 * c + 3),
                        )

                def emit_add(pt, d0, va, acc, first):
                    if first:
                        nc.vector.tensor_copy(acc[:], pt[:, d0:d0 + 1024])
                    else:
                        nc.vector.tensor_add(
                            acc[:, va:1024], acc[:, va:1024],
                            pt[:, d0 + va:d0 + 1024])

                for g in range(QG):
                    q0 = 1024 * g
                    o_ps = ps_o.tile([P, 1024], f32)
                    acc = accp.tile([P, 1024], f16, tag="acc", name="acc")
                    nkv = 8 * (g + 1)
                    for k in range(nkv):
                        sT = ring.tile([P, 1024], f32, tag="s", name="s")
                        va = emit_score(g, k, sT, 0)
                        emit_mask(g, k, sT, 0, va)
                        pt = ptp.tile([P, 1024], f16, tag="pt", name="pt")
                        nc.scalar.activation(
                            pt[:, va:1024], sT[:, va:1024], EXP, scale=SCALE)
                        emit_add(pt, 0, va, acc, k == 0)
                        emit_pv(g, k, pt, 0, va, o_ps)

                    # ---- epilogue: free o_ps ASAP (copy out on DVE), then
                    # run the denominator chain in a ring slot off the
                    # critical path
                    ocu = ocup.tile([P, 1024], f32, tag="ocu", name="ocu")
                    nc.vector.tensor_copy(ocu[:], o_ps[:])
                    ep = ring.tile([P, 1024], f32, tag="s", name="s")
                    for c in range(2):
                        nc.tensor.matmul(
                            ep[0:1, 512 * c:512 * (c + 1)],
                            lhsT=ones_col, rhs=acc[:, 512 * c:512 * (c + 1)],
                            start=True, stop=True,
                        )
                    recipT = recipp.tile([1, 1024], f32, tag="recipT",
                                         name="recipT")
                    nc.vector.reciprocal_approx_fast(
                        recipT[:], ep[0:1, 0:1024])
                    recipH = recipp.tile([1, 1024], f16, tag="recipH",
                                         name="recipH")
                    nc.vector.tensor_copy(recipH[:], recipT[:])
                    for c in range(2):
                        nc.tensor.matmul(
                            ep[:, 512 * c:512 * (c + 1)],
                            lhsT=ones_row,
                            rhs=recipH[0:1, 512 * c:512 * (c + 1)],
                            start=True, stop=True,
                        )
                    ocn = ocnp.tile([P, 1024], f32, tag="ocn", name="ocn")
                    nc.vector.tensor_mul(ocn[:], ocu[:], ep[:])
                    nc.sync.dma_start(out=outT[:, q0:q0 + 1024], in_=ocn[:])

    nc.finalize()
    return nc


def _get_nc():
    global _NC
    if _NC is None:
        _NC = build_program()
    return _NC


def make_in_maps(x, Wq, Wk, Wv):
    xh = np.asarray(x, dtype=np.float32).astype(np.float16)
    # pack [C, HS] x3 -> [p, i, j, d]: Wall[p, i, j, :] = W_i[j*128+p, :]
    ws = np.stack([np.asarray(w, dtype=np.float32).astype(np.float16)
                   for w in (Wq, Wk, Wv)])            # [3, C, HS]
    wall = np.ascontiguousarray(
        ws.reshape(3, CCH, P, HS).transpose(2, 0, 1, 3).reshape(P, -1))
    aux = build_aux()
    return [
        {
            "xT": np.ascontiguousarray(xh[b].T),
            "Wall": wall,
            "aux": aux,
        }
        for b in range(NCORES)
    ]


def kernel(x, Wq, Wk, Wv):
    assert x.shape == (B, T, C) and Wq.shape == (C, HS)
    nc = _get_nc()
    in_maps = make_in_maps(x, Wq, Wk, Wv)
    res = run_bass_kernel_spmd(nc, in_maps, list(range(NCORES)))
    return np.stack([np.ascontiguousarray(res.results[b]["outT"].T)
                     for b in range(NCORES)])
